# revision 37
# baseline (speedup 1.0000x reference)
"""Trainium2 Bass kernel for AttentionWeightedCELoss.

Full inputs in, full (scalar) output out. Sharding: data-parallel over the
batch dim — core b processes batch b. Each core computes per-class partial
sums; the tiny per-class partials are combined on the host into the final
scalar loss.

Device algorithm per core (pixels N = 512*512, classes C = 10), bf16 inputs:
  - class-expanded super-tiles [80 = 10 classes x 8 pixel-blocks, 8192 px]
  - ACT: E = exp(S); POOL: ES = E*S
  - PE selector matmuls (bf16 operands, f32 PSUM) collapse the class dim ->
    stacked per-pixel [128,512] PSUM tiles (sumexp / dot) per super-tile
    (stacked partition p = r*TPS + t2, r-major)
  - ACT: lse = log(sumexp), isx = exp(-lse); DVE: ent = lse - dot*isx
  - per-class masked sums via the max-telescope trick: for a per-pixel
    quantity x >= 0 and V = x + K*t (K > max x), sum_pix max(V, K*c) =
    sum_{t>=c}(x + K*t) + K*c*N_{<c}, so consecutive-threshold differences
    recover sum_{t==c} x exactly. These run as plain tensor_scalar(max)
    ops with accum_out at the 2x (f32) / 4x (bf16) DVE rates.
  - target-class logit sums (G) via fused scalar_tensor_tensor in the
    class-expanded layout.
"""

import numpy as np
import ml_dtypes

import concourse.bass as bass
import concourse.bacc as bacc
import concourse.tile as tile
from concourse import mybir
from concourse.bass_utils import run_bass_kernel_spmd

F32 = mybir.dt.float32
BF16 = mybir.dt.bfloat16
NP_BF16 = np.dtype(ml_dtypes.bfloat16)

B, C, H, W = 8, 10, 512, 512
N = H * W            # 262144 pixels per batch/core
R = 8                # pixel blocks stacked with classes on partitions
P = C * R            # 80 partitions in class-expanded layout
Q = 512              # tile width (pixels per block per tile)
ILEN = N // R        # 32768 pixels per block
NT = N // (R * Q)    # 64 tiles
TPS = 16             # tiles per super-tile (8*16 = 128 stacked partitions)
NST = NT // TPS      # 4 super-tiles
SW = TPS * Q         # super-tile width per block row (8192 pixels)
NC11 = C + 1         # telescope thresholds c = 0..10

K = 16.0             # telescope separation constant (> max base value)
BOFF = 4.0           # offset making lse + BOFF > 0

_CACHE = {}


def _patch_act_tables():
    # Put the combined exp+ln set first so the table-load inserter resolves
    # both Exp and Ln to one set (avoids ~1.3us reloads between them).
    import concourse.bacc as _bacc
    import concourse.mybir as _mybir
    orig = _bacc.get_activation_tables
    def filtered(arch, _orig=orig):
        # keep set order/indices intact; just make the combined set the
        # only one advertising Exp and Ln so the chooser picks it for both
        tabs = _orig(arch)
        key = "natural_log_exp_and_others"
        if key not in tabs:
            return tabs
        drop = {_mybir.ActivationFunctionType.Exp,
                _mybir.ActivationFunctionType.Ln}
        out = {}
        for k, v in tabs.items():
            out[k] = set(v) if k == key else (set(v) - drop)
        return out
    _bacc.get_activation_tables = filtered


_patch_act_tables()


def _consts():
    # SS: sliding selector for stacking (r-major: block r, tile t2 ->
    # stacked partition r*TPS + t2). SS[(c,r), i] = 1 iff i == 128 + TPS*r;
    # lhsT slice SS[:, 128-t2 : 256-t2] selects columns m = TPS*r + t2 and
    # sums over classes.
    ss = np.zeros((P, 256), NP_BF16)
    for c in range(C):
        for r in range(R):
            ss[c * R + r, 128 + TPS * r] = 1.0
    iotac = np.zeros((P, 1), np.float32)
    for c in range(C):
        iotac[c * R:(c + 1) * R, 0] = float(c)
    return ss, iotac


def _build():
    nc = bacc.Bacc(None, target_bir_lowering=False)
    logits_d = nc.declare_dram_parameter("logits", [C, N], BF16, isOutput=False)
    tgt_d = nc.declare_dram_parameter("tgt", [N], BF16, isOutput=False)
    ss_d = nc.declare_dram_parameter("ss", [P, 256], BF16, isOutput=False)
    iotac_d = nc.declare_dram_parameter("iotac", [P, 1], F32, isOutput=False)
    # acc[0] = t-telescope (counts), acc[1] = ent-telescope, acc[2] =
    # lse-telescope (each [128, NST*11] used), acc[3] = G sums ([80, 2*NST])
    acc_d = nc.declare_dram_parameter("acc", [4, 128, 64], F32, isOutput=True)

    # views (pixel index = r*ILEN + tile*Q + q within a class plane; the 16
    # tiles of a super-tile are one contiguous 8192-element run per block)
    lg = logits_d.rearrange("c (r st w) -> (c r) st w", r=R, w=SW)  # [80,4,8192]
    tst = tgt_d.rearrange("(r st t q) -> r st t q", r=R, st=NST, q=Q)

    with tile.TileContext(nc) as tc:
        with (
            tc.tile_pool(name="const", bufs=1) as constp,
            tc.tile_pool(name="sst", bufs=2) as sstp,
            tc.tile_pool(name="est", bufs=2) as estp,
            tc.tile_pool(name="tstk", bufs=2) as tstkp,
            tc.tile_pool(name="tball", bufs=2) as tballp,
            tc.tile_pool(name="dense", bufs=2) as densep,
            tc.tile_pool(name="scrap", bufs=2) as scrapp,
            tc.tile_pool(name="accp", bufs=1) as accp,
            tc.tile_pool(name="psum", bufs=3, space=bass.MemorySpace.PSUM) as psump,
        ):
            ss_t = constp.tile([P, 256], BF16, tag="ss")
            nc.sync.dma_start(ss_t[:], ss_d[:])
            iota_t = constp.tile([P, 1], F32, tag="iota")
            nc.sync.dma_start(iota_t[:], iotac_d[:])

            accM = accp.tile([128, 64], F32, tag="accM")
            accE = accp.tile([128, 64], F32, tag="accE")
            accB = accp.tile([128, 64], F32, tag="accB")
            accG = accp.tile([128, 64], F32, tag="accG")
            for a in (accM, accE, accB, accG):
                nc.vector.memset(a[:], 0.0)

            for st in range(NST):
                # --- stacked targets (r-major: p = r*TPS + t2) ---
                t_st = tstkp.tile([128, Q], BF16, tag="tst")
                nc.sync.dma_start(t_st[:], tst[:, st])
                # t_b_all[(c,r), t2, q] = t_st[r*TPS+t2, q]: flat element
                # orders match -> one partition->free fold DMA per class
                t_b_all = tballp.tile([P, TPS, Q], BF16, tag="tball")
                for c in range(C):
                    nc.sync.dma_start(t_b_all[c * R:(c + 1) * R], t_st[:])

                # --- class-expanded phase ---
                s_st = sstp.tile([P, SW], BF16, tag="sst")
                nc.sync.dma_start(s_st[:], lg[:, st, :])
                # split the big elementwise ops so the consumers (PE
                # matmuls) can start before the whole super-tile is done
                e_st = estp.tile([P, SW], BF16, tag="est")
                for h in range(2):
                    hs = slice(h * (SW // 2), (h + 1) * (SW // 2))
                    nc.scalar.activation(e_st[:, hs], s_st[:, hs],
                                         mybir.ActivationFunctionType.Exp)
                es_st = estp.tile([P, SW], BF16, tag="esst")
                for qq in range(4):
                    qs = slice(qq * (SW // 4), (qq + 1) * (SW // 4))
                    nc.gpsimd.tensor_mul(es_st[:, qs], e_st[:, qs],
                                         s_st[:, qs])

                # --- G sums (class-expanded, fused STT, two halves);
                # high priority: they only need s_st + t_b_all and should
                # fill the DVE idle window while sumexp/dot are in flight
                tb_flat = t_b_all[:].rearrange("p t q -> p (t q)")
                with tc.high_priority():
                    for h in range(2):
                        hs = slice(h * (SW // 2), (h + 1) * (SW // 2))
                        gsc = scrapp.tile([P, SW // 2], BF16, tag="scrapg")
                        nc.vector.scalar_tensor_tensor(
                            gsc[:], tb_flat[:, hs], iota_t[:, 0:1],
                            s_st[:, hs],
                            mybir.AluOpType.is_equal, mybir.AluOpType.mult,
                            accum_out=accG[:P, 2 * st + h:2 * st + h + 1])

                se_ps = psump.tile([128, Q], F32, tag="sumexp")
                dot_ps = psump.tile([128, Q], F32, tag="dot")
                for t2 in range(TPS):
                    sel = ss_t[:, 128 - t2:256 - t2]
                    first = t2 == 0
                    last = t2 == TPS - 1
                    sl = slice(t2 * Q, (t2 + 1) * Q)
                    nc.tensor.matmul(se_ps[:], sel, e_st[:, sl],
                                     start=first, stop=last)
                    nc.tensor.matmul(dot_ps[:], sel, es_st[:, sl],
                                     start=first, stop=last)

                # --- dense per-pixel phase on stacked [128, 512] ---
                lse_st = densep.tile([128, Q], F32, tag="lse")
                nc.scalar.activation(lse_st[:], se_ps[:],
                                     mybir.ActivationFunctionType.Ln)
                isx_st = densep.tile([128, Q], F32, tag="isx")
                nc.scalar.activation(isx_st[:], lse_st[:],
                                     mybir.ActivationFunctionType.Exp,
                                     scale=-1.0)
                # lseKt = lse + K*t (uniform f32 operands: mixed-dtype
                # scalar_tensor_tensor misreads on hardware)
                t_f = densep.tile([128, Q], F32, tag="tf")
                nc.vector.tensor_copy(t_f[:], t_st[:])
                lsekt = densep.tile([128, Q], F32, tag="lsekt")
                nc.vector.scalar_tensor_tensor(
                    lsekt[:], t_f[:], K, lse_st[:],
                    mybir.AluOpType.mult, mybir.AluOpType.add)
                ratio_st = densep.tile([128, Q], F32, tag="ratio")
                nc.vector.tensor_mul(ratio_st[:], dot_ps[:], isx_st[:])
                # vE = ent + K*t = lseKt - ratio
                ve_st = densep.tile([128, Q], F32, tag="ve")
                nc.vector.tensor_sub(ve_st[:], lsekt[:], ratio_st[:])

                # --- max-telescope accumulations ---
                for c in range(NC11):
                    col = st * NC11 + c
                    sc = scrapp.tile([128, Q], BF16, tag="scrapm")
                    nc.vector.tensor_scalar(
                        sc[:], t_st[:], float(c), None,
                        mybir.AluOpType.max, mybir.AluOpType.add,
                        accum_out=accM[:, col:col + 1])
                    sc = scrapp.tile([128, Q], F32, tag="scrape")
                    nc.vector.tensor_scalar(
                        sc[:], ve_st[:], K * c, None,
                        mybir.AluOpType.max, mybir.AluOpType.add,
                        accum_out=accE[:, col:col + 1])
                    sc = scrapp.tile([128, Q], F32, tag="scrapb")
                    nc.vector.tensor_scalar(
                        sc[:], lsekt[:], K * c - BOFF, None,
                        mybir.AluOpType.max, mybir.AluOpType.add,
                        accum_out=accB[:, col:col + 1])

            nc.sync.dma_start(acc_d[0], accM[:])
            nc.sync.dma_start(acc_d[1], accE[:])
            nc.sync.dma_start(acc_d[2], accB[:])
            nc.sync.dma_start(acc_d[3], accG[:])

    nc.compile()
    return nc


def kernel(logits, targets):
    logits_b = np.asarray(logits).astype(NP_BF16)
    tgt_b = np.asarray(targets).astype(NP_BF16)

    if "nc" not in _CACHE:
        _CACHE["nc"] = _build()
    nc = _CACHE["nc"]

    ss, iotac = _consts()
    in_maps = []
    for b in range(B):
        in_maps.append({
            "logits": np.ascontiguousarray(logits_b[b].reshape(C, N)),
            "tgt": np.ascontiguousarray(tgt_b[b].reshape(N)),
            "ss": ss,
            "iotac": iotac,
        })
    res = run_bass_kernel_spmd(nc, in_maps, list(range(B)))

    MT = np.zeros(NC11, np.float64)
    ME = np.zeros(NC11, np.float64)
    MB = np.zeros(NC11, np.float64)
    accG = np.zeros(C, np.float64)
    for b in range(B):
        acc = np.asarray(res.results[b]["acc"], np.float64)  # [4,128,64]
        for st in range(NST):
            cols = acc[:, :, st * NC11:(st + 1) * NC11]
            MT += cols[0].sum(axis=0)
            ME += cols[1].sum(axis=0)
            MB += cols[2].sum(axis=0)
        g = acc[3, :P, :2 * NST].reshape(C, R, 2 * NST)
        accG += g.sum(axis=(1, 2))

    npix_total = float(B * N)
    cr = np.arange(NC11, dtype=np.float64)
    # t-telescope: MT_c = sum max(t, c); N_{<c+1} = MT_{c+1} - MT_c
    N_lt = np.zeros(C + 2, np.float64)       # N_lt[c] = #pixels with t < c
    for c in range(C):
        N_lt[c + 1] = MT[c + 1] - MT[c]
    N_lt[C + 1] = npix_total
    counts = N_lt[1:C + 1] - N_lt[0:C]       # per class 0..9
    n_valid = N_lt[C]
    # T_ge[c] = sum_{t>=c} t = MT_c - c*N_{<c}
    T_ge = MT - cr * N_lt[:NC11]
    # ent-telescope: ME_c = Ent_ge_c + K*T_ge_c + K*c*N_{<c}
    Ent_ge = ME - K * T_ge - K * cr * N_lt[:NC11]
    accE_c = Ent_ge[0:C] - Ent_ge[1:C + 1]
    # lse-telescope: MB_c = sum_{t>=c}(lse + K*t) + (K*c - BOFF)*N_{<c}
    L_ge = MB - K * T_ge - (K * cr - BOFF) * N_lt[:NC11]
    accB_c = L_ge[0:C] - L_ge[1:C + 1]

    ce_sum = accB_c - accG
    has = (counts > 0) & (n_valid > 0)
    w_base = np.where(has, (n_valid - counts) / max(n_valid, 1.0), 0.0)
    ent_mean = np.where(counts > 0, accE_c / np.maximum(counts, 1.0), 0.0)
    w = w_base * (1.0 + 0.5 * ent_mean)
    loss = (w * ce_sum).sum() / (n_valid + 1e-6)
    return np.float32(loss)


# revision 43
# speedup vs baseline: 1.2770x; 1.2770x over previous
"""Trainium2 Bass kernel for AttentionWeightedCELoss.

Full inputs in, full (scalar) output out. Sharding: data-parallel over the
batch dim — core b processes batch b. Each core computes per-class partial
sums; the tiny per-class partials are combined on the host into the final
scalar loss.

Device algorithm per core (pixels N = 512*512, classes C = 10), bf16 inputs:
  - class-expanded super-tiles [80 = 10 classes x 8 pixel-blocks, 8192 px]
  - ACT: E = exp(S); POOL: ES = E*S
  - PE selector matmuls (bf16 operands, f32 PSUM) collapse the class dim ->
    stacked per-pixel [128,512] PSUM tiles (sumexp / dot) per super-tile
    (stacked partition p = r*TPS + t2, r-major)
  - ACT: lse = log(sumexp), isx = exp(-lse); DVE: ent = lse - dot*isx
  - per-class masked sums via the max-telescope trick: for a per-pixel
    quantity x >= 0 and V = x + K*t (K > max x), sum_pix max(V, K*c) =
    sum_{t>=c}(x + K*t) + K*c*N_{<c}, so consecutive-threshold differences
    recover sum_{t==c} x exactly. These run as plain tensor_scalar(max)
    ops with accum_out at the 2x (f32) / 4x (bf16) DVE rates.
  - target-class logit sums (G) via fused scalar_tensor_tensor in the
    class-expanded layout.
"""

import numpy as np
import ml_dtypes

import concourse.bass as bass
import concourse.bacc as bacc
import concourse.tile as tile
from concourse import mybir
from concourse.bass_utils import run_bass_kernel_spmd

F32 = mybir.dt.float32
BF16 = mybir.dt.bfloat16
NP_BF16 = np.dtype(ml_dtypes.bfloat16)

B, C, H, W = 8, 10, 512, 512
N = H * W            # 262144 pixels per batch/core
R = 8                # pixel blocks stacked with classes on partitions
P = C * R            # 80 partitions in class-expanded layout
Q = 512              # tile width (pixels per block per tile)
ILEN = N // R        # 32768 pixels per block
NT = N // (R * Q)    # 64 tiles
TPS = 16             # tiles per super-tile (8*16 = 128 stacked partitions)
NST = NT // TPS      # 4 super-tiles
SW = TPS * Q         # super-tile width per block row (8192 pixels)
NC11 = C + 1         # telescope thresholds c = 0..10

K = 16.0             # telescope separation constant (> max base value)
BOFF = 4.0           # offset making lse + BOFF > 0

_CACHE = {}


def _patch_act_tables():
    # Put the combined exp+ln set first so the table-load inserter resolves
    # both Exp and Ln to one set (avoids ~1.3us reloads between them).
    import concourse.bacc as _bacc
    import concourse.mybir as _mybir
    orig = _bacc.get_activation_tables
    def filtered(arch, _orig=orig):
        # keep set order/indices intact; just make the combined set the
        # only one advertising Exp and Ln so the chooser picks it for both
        tabs = _orig(arch)
        key = "natural_log_exp_and_others"
        if key not in tabs:
            return tabs
        drop = {_mybir.ActivationFunctionType.Exp,
                _mybir.ActivationFunctionType.Ln}
        out = {}
        for k, v in tabs.items():
            out[k] = set(v) if k == key else (set(v) - drop)
        return out
    _bacc.get_activation_tables = filtered


_patch_act_tables()


def _consts():
    # SS: sliding selector for stacking (r-major: block r, tile t2 ->
    # stacked partition r*TPS + t2). SS[(c,r), i] = 1 iff i == 128 + TPS*r;
    # lhsT slice SS[:, 128-t2 : 256-t2] selects columns m = TPS*r + t2 and
    # sums over classes.
    ss = np.zeros((P, 256), NP_BF16)
    for c in range(C):
        for r in range(R):
            ss[c * R + r, 128 + TPS * r] = 1.0
    iotac = np.zeros((P, 1), np.float32)
    for c in range(C):
        iotac[c * R:(c + 1) * R, 0] = float(c)
    return ss, iotac


def _build():
    nc = bacc.Bacc(None, target_bir_lowering=False)
    logits_d = nc.declare_dram_parameter("logits", [C, N], BF16, isOutput=False)
    tgt_d = nc.declare_dram_parameter("tgt", [N], BF16, isOutput=False)
    ss_d = nc.declare_dram_parameter("ss", [P, 256], BF16, isOutput=False)
    iotac_d = nc.declare_dram_parameter("iotac", [P, 1], F32, isOutput=False)
    # acc[0] = t-telescope (counts), acc[1] = ent-telescope, acc[2] =
    # lse-telescope (each [128, NST*11] used), acc[3] = G sums ([80, 2*NST])
    acc_d = nc.declare_dram_parameter("acc", [4, 128, 64], F32, isOutput=True)

    # views (pixel index = r*ILEN + tile*Q + q within a class plane; the 16
    # tiles of a super-tile are one contiguous 8192-element run per block)
    lg = logits_d.rearrange("c (r st w) -> (c r) st w", r=R, w=SW)  # [80,4,8192]
    tst = tgt_d.rearrange("(r st t q) -> r st t q", r=R, st=NST, q=Q)

    with tile.TileContext(nc) as tc:
        with (
            tc.tile_pool(name="const", bufs=1) as constp,
            tc.tile_pool(name="sst", bufs=2) as sstp,
            tc.tile_pool(name="est", bufs=2) as estp,
            tc.tile_pool(name="tstk", bufs=2) as tstkp,
            tc.tile_pool(name="tball", bufs=2) as tballp,
            tc.tile_pool(name="dense", bufs=2) as densep,
            tc.tile_pool(name="scrap", bufs=2) as scrapp,
            tc.tile_pool(name="accp", bufs=1) as accp,
            tc.tile_pool(name="psum", bufs=3, space=bass.MemorySpace.PSUM) as psump,
        ):
            ss_t = constp.tile([P, 256], BF16, tag="ss")
            nc.sync.dma_start(ss_t[:], ss_d[:])
            iota_t = constp.tile([P, 1], F32, tag="iota")
            nc.sync.dma_start(iota_t[:], iotac_d[:])

            accM = accp.tile([128, 64], F32, tag="accM")
            accE = accp.tile([128, 64], F32, tag="accE")
            accB = accp.tile([128, 64], F32, tag="accB")
            accG = accp.tile([128, 64], F32, tag="accG")
            for a in (accM, accE, accB, accG):
                nc.vector.memset(a[:], 0.0)

            for st in range(NST):
                # --- stacked targets (r-major: p = r*TPS + t2) ---
                t_st = tstkp.tile([128, Q], BF16, tag="tst")
                nc.sync.dma_start(t_st[:], tst[:, st])
                # t_b_all[(c,r), t2, q] = t_st[r*TPS+t2, q]: flat element
                # orders match -> one partition->free fold DMA per class
                t_b_all = tballp.tile([P, TPS, Q], BF16, tag="tball")
                for c in range(C):
                    nc.sync.dma_start(t_b_all[c * R:(c + 1) * R], t_st[:])

                # --- class-expanded phase ---
                # finer chunks on the first super-tile shorten the pipeline
                # fill (everything downstream waits on its exp chain)
                nspl = 4
                s_st = sstp.tile([P, SW], BF16, tag="sst")
                for h in range(nspl):
                    hs = slice(h * (SW // nspl), (h + 1) * (SW // nspl))
                    nc.sync.dma_start(s_st[:, hs], lg[:, st, hs])
                e_st = estp.tile([P, SW], BF16, tag="est")
                for h in range(2 * nspl):
                    hs = slice(h * (SW // (2 * nspl)),
                               (h + 1) * (SW // (2 * nspl)))
                    nc.scalar.activation(e_st[:, hs], s_st[:, hs],
                                         mybir.ActivationFunctionType.Exp)
                es_st = estp.tile([P, SW], BF16, tag="esst")
                # st0's E*S on DVE (2x bf16): DVE is idle during pipeline
                # fill and the slower POOL op would sit on the critical path
                es_eng = nc.gpsimd
                for qq in range(4):
                    qs = slice(qq * (SW // 4), (qq + 1) * (SW // 4))
                    es_eng.tensor_mul(es_st[:, qs], e_st[:, qs],
                                      s_st[:, qs])

                # --- G sums (class-expanded, fused STT, two halves);
                # high priority: they only need s_st + t_b_all and should
                # fill the DVE idle window while sumexp/dot are in flight
                tb_flat = t_b_all[:].rearrange("p t q -> p (t q)")
                with tc.high_priority():
                    for h in range(2):
                        hs = slice(h * (SW // 2), (h + 1) * (SW // 2))
                        gsc = scrapp.tile([P, SW // 2], BF16, tag="scrapg")
                        nc.vector.scalar_tensor_tensor(
                            gsc[:], tb_flat[:, hs], iota_t[:, 0:1],
                            s_st[:, hs],
                            mybir.AluOpType.is_equal, mybir.AluOpType.mult,
                            accum_out=accG[:P, 2 * st + h:2 * st + h + 1])

                se_ps = psump.tile([128, Q], F32, tag="sumexp")
                dot_ps = psump.tile([128, Q], F32, tag="dot")
                for t2 in range(TPS):
                    sel = ss_t[:, 128 - t2:256 - t2]
                    first = t2 == 0
                    last = t2 == TPS - 1
                    sl = slice(t2 * Q, (t2 + 1) * Q)
                    nc.tensor.matmul(se_ps[:], sel, e_st[:, sl],
                                     start=first, stop=last)
                    nc.tensor.matmul(dot_ps[:], sel, es_st[:, sl],
                                     start=first, stop=last)

                # --- dense per-pixel phase on stacked [128, 512] ---
                lse_st = densep.tile([128, Q], F32, tag="lse")
                nc.scalar.activation(lse_st[:], se_ps[:],
                                     mybir.ActivationFunctionType.Ln)
                isx_st = densep.tile([128, Q], F32, tag="isx")
                nc.scalar.activation(isx_st[:], lse_st[:],
                                     mybir.ActivationFunctionType.Exp,
                                     scale=-1.0)
                # lseKt = lse + K*t (uniform f32 operands: mixed-dtype
                # scalar_tensor_tensor misreads on hardware)
                t_f = densep.tile([128, Q], F32, tag="tf")
                nc.gpsimd.tensor_copy(t_f[:], t_st[:])
                lsekt = densep.tile([128, Q], F32, tag="lsekt")
                nc.vector.scalar_tensor_tensor(
                    lsekt[:], t_f[:], K, lse_st[:],
                    mybir.AluOpType.mult, mybir.AluOpType.add,
                    accum_out=accB[:, st * NC11:st * NC11 + 1])
                ratio_st = densep.tile([128, Q], F32, tag="ratio")
                nc.vector.tensor_mul(ratio_st[:], dot_ps[:], isx_st[:])
                # vE = ent + K*t = lseKt - ratio
                ve_st = densep.tile([128, Q], F32, tag="ve")
                nc.vector.tensor_sub(ve_st[:], lsekt[:], ratio_st[:])

                # --- max-telescope accumulations (c=0 sums are folded
                # into the lsekt/ve producers' accum_out above) ---
                for c in range(NC11):
                    col = st * NC11 + c
                    sc = scrapp.tile([128, Q], BF16, tag="scrapm")
                    nc.vector.tensor_scalar(
                        sc[:], t_st[:], float(c), None,
                        mybir.AluOpType.max, mybir.AluOpType.add,
                        accum_out=accM[:, col:col + 1])
                    sc = scrapp.tile([128, Q], F32, tag="scrape")
                    nc.vector.tensor_scalar(
                        sc[:], ve_st[:], K * c, None,
                        mybir.AluOpType.max, mybir.AluOpType.add,
                        accum_out=accE[:, col:col + 1])
                    if c == 0:
                        continue
                    sc = scrapp.tile([128, Q], F32, tag="scrapb")
                    nc.vector.tensor_scalar(
                        sc[:], lsekt[:], K * c - BOFF, None,
                        mybir.AluOpType.max, mybir.AluOpType.add,
                        accum_out=accB[:, col:col + 1])

            nc.sync.dma_start(acc_d[0], accM[:])
            nc.sync.dma_start(acc_d[1], accE[:])
            nc.sync.dma_start(acc_d[2], accB[:])
            nc.sync.dma_start(acc_d[3], accG[:])

    nc.compile()
    return nc


def kernel(logits, targets):
    logits_b = np.asarray(logits).astype(NP_BF16)
    tgt_b = np.asarray(targets).astype(NP_BF16)

    if "nc" not in _CACHE:
        _CACHE["nc"] = _build()
    nc = _CACHE["nc"]

    ss, iotac = _consts()
    in_maps = []
    for b in range(B):
        in_maps.append({
            "logits": np.ascontiguousarray(logits_b[b].reshape(C, N)),
            "tgt": np.ascontiguousarray(tgt_b[b].reshape(N)),
            "ss": ss,
            "iotac": iotac,
        })
    res = run_bass_kernel_spmd(nc, in_maps, list(range(B)))

    MT = np.zeros(NC11, np.float64)
    ME = np.zeros(NC11, np.float64)
    MB = np.zeros(NC11, np.float64)
    accG = np.zeros(C, np.float64)
    for b in range(B):
        acc = np.asarray(res.results[b]["acc"], np.float64)  # [4,128,64]
        for st in range(NST):
            cols = acc[:, :, st * NC11:(st + 1) * NC11]
            MT += cols[0].sum(axis=0)
            ME += cols[1].sum(axis=0)
            MB += cols[2].sum(axis=0)
        g = acc[3, :P, :2 * NST].reshape(C, R, 2 * NST)
        accG += g.sum(axis=(1, 2))

    npix_total = float(B * N)
    cr = np.arange(NC11, dtype=np.float64)
    # t-telescope: MT_c = sum max(t, c); N_{<c+1} = MT_{c+1} - MT_c
    N_lt = np.zeros(C + 2, np.float64)       # N_lt[c] = #pixels with t < c
    for c in range(C):
        N_lt[c + 1] = MT[c + 1] - MT[c]
    N_lt[C + 1] = npix_total
    counts = N_lt[1:C + 1] - N_lt[0:C]       # per class 0..9
    n_valid = N_lt[C]
    # T_ge[c] = sum_{t>=c} t = MT_c - c*N_{<c}
    T_ge = MT - cr * N_lt[:NC11]
    # ent-telescope: ME_c = Ent_ge_c + K*T_ge_c + K*c*N_{<c}
    Ent_ge = ME - K * T_ge - K * cr * N_lt[:NC11]
    accE_c = Ent_ge[0:C] - Ent_ge[1:C + 1]
    # lse-telescope: MB_c = sum_{t>=c}(lse + K*t) + (K*c - BOFF)*N_{<c}
    L_ge = MB - K * T_ge - (K * cr - BOFF) * N_lt[:NC11]
    accB_c = L_ge[0:C] - L_ge[1:C + 1]

    ce_sum = accB_c - accG
    has = (counts > 0) & (n_valid > 0)
    w_base = np.where(has, (n_valid - counts) / max(n_valid, 1.0), 0.0)
    ent_mean = np.where(counts > 0, accE_c / np.maximum(counts, 1.0), 0.0)
    w = w_base * (1.0 + 0.5 * ent_mean)
    loss = (w * ce_sum).sum() / (n_valid + 1e-6)
    return np.float32(loss)
